# revision 5
# baseline (speedup 1.0000x reference)
"""Causal GQA attention on 8 TRN2 NeuronCores (head-sharded, no collectives).

Problem: NUM_TOKENS=2048, NUM_HEADS=32, HEAD_DIM=128, NUM_KV_HEADS=8, causal.
Sharding: core i holds KV head i and its 4 query heads (GQA group stays
together). Each core runs an independent flash-attention-style kernel:
  ST[k,q] = K @ Q^T  (bf16 matmuls, PSUM f32)
  PT      = exp(SCALE * ST)           (ACT, no max-subtraction: scores ~N(0,1))
  O[q, d+1] += PT_block^T @ [V | 1]   (ones column yields softmax denominators)
  out     = O[:, :d] / O[:, d]
"""

import numpy as np

import concourse.bass as bass
import concourse.bacc as bacc
import concourse.mybir as mybir
from concourse.tile import TileContext
from concourse.masks import make_identity
from concourse.bass_utils import run_bass_kernel_spmd

T = 2048          # tokens
D = 128           # head dim
HEADS = 4         # query heads per core
N_CORES = 8
W = 512           # q-chunk width
NKT = T // 128    # 16 k-tiles
SCALE = D ** -0.5
F32 = mybir.dt.float32
BF16 = mybir.dt.bfloat16
EXP = mybir.ActivationFunctionType.Exp


def build_attention_nc():
    nc = bacc.Bacc("TRN2", target_bir_lowering=False, debug=False)

    q_in = nc.declare_dram_parameter("query", [T, HEADS * D], F32, isOutput=False)
    k_in = nc.declare_dram_parameter("key", [T, D], F32, isOutput=False)
    v_in = nc.declare_dram_parameter("value", [T, D], F32, isOutput=False)
    out = nc.declare_dram_parameter("out", [T, HEADS * D], F32, isOutput=True)

    # persistent SBUF tensors
    qt_sb = nc.alloc_sbuf_tensor("qt_sb", [128, HEADS, T], BF16).ap()   # [d, h, q]
    kt_sb = nc.alloc_sbuf_tensor("kt_sb", [128, NKT, 128], BF16).ap()   # [d, kt, k]
    v_aug = nc.alloc_sbuf_tensor("v_aug", [128, NKT, 132], BF16).ap()   # [k, kt, d+1]
    ident = nc.alloc_sbuf_tensor("ident", [128, 128], BF16).ap()

    with TileContext(nc) as tc:
        make_identity(nc, ident)
        nc.gpsimd.memset(v_aug[:, :, 128:129], 1.0)

        # ---- phase 0: load, cast to bf16, transpose Q and K ----
        with (
            tc.tile_pool(name="ldf32", bufs=3) as ldf32,
            tc.tile_pool(name="ldbf", bufs=3) as ldbf,
            tc.tile_pool(name="trps", bufs=4, space="PSUM") as trps,
        ):
            for j in range(NKT):
                r0 = j * 128
                kf = ldf32.tile([128, 128], F32, tag="kf")
                nc.sync.dma_start(out=kf[:], in_=k_in[r0 : r0 + 128, :])
                kb = ldbf.tile([128, 128], BF16, tag="kb")
                nc.vector.tensor_copy(kb[:], kf[:])
                kt_ps = trps.tile([128, 128], BF16, tag="tr")
                nc.tensor.transpose(kt_ps[:], kb[:], ident[:])
                nc.vector.tensor_copy(kt_sb[:, j, :], kt_ps[:])

                vf = ldf32.tile([128, 128], F32, tag="vf")
                nc.sync.dma_start(out=vf[:], in_=v_in[r0 : r0 + 128, :])
                nc.vector.tensor_copy(v_aug[:, j, 0:128], vf[:])

            for t in range(NKT):
                r0 = t * 128
                qf = ldf32.tile([128, 512], F32, tag="qf")
                nc.sync.dma_start(out=qf[:], in_=q_in[r0 : r0 + 128, :])
                qb = ldbf.tile([128, 512], BF16, tag="qb")
                nc.vector.tensor_copy(qb[:], qf[:])
                for h in range(HEADS):
                    qt_ps = trps.tile([128, 128], BF16, tag="tr")
                    nc.tensor.transpose(qt_ps[:], qb[:, h * 128 : (h + 1) * 128], ident[:])
                    nc.vector.tensor_copy(qt_sb[:, h, r0 : r0 + 128], qt_ps[:])

        # ---- phase 1: attention ----
        with (
            tc.tile_pool(name="st", bufs=2, space="PSUM") as stp,
            tc.tile_pool(name="ops", bufs=1, space="PSUM") as ops,
            tc.tile_pool(name="pt", bufs=3) as ptp,
            tc.tile_pool(name="osb", bufs=4) as osb,
            tc.tile_pool(name="rp", bufs=4) as rp,
        ):
            for h in range(HEADS):
                for c in range(T // W):
                    q0 = c * W
                    n_kt = 4 * c + 4  # causal: k-tiles 0 .. 4c+3
                    o_ts = [
                        ops.tile([128, 2, 130], F32, tag="o01", name="o01"),
                        ops.tile([128, 2, 130], F32, tag="o23", name="o23"),
                    ]
                    nc.vector.memset(o_ts[0][:], 0.0)
                    nc.vector.memset(o_ts[1][:], 0.0)

                    ks = list(range(n_kt))
                    for g0 in range(0, n_kt, 3):
                        grp = ks[g0 : g0 + 3]
                        n = len(grp)
                        st = stp.tile([128, 3, 512], F32, tag="st")
                        pt = ptp.tile([128, 3, 512], BF16, tag="pt")
                        for j2, j in enumerate(grp):
                            off = max(0, (j - 4 * c) * 128)
                            nc.tensor.matmul(
                                st[:, j2, off:512],
                                lhsT=kt_sb[:, j, :],
                                rhs=qt_sb[:, h, q0 + off : q0 + W],
                                start=True,
                                stop=True,
                            )
                        nc.scalar.activation(
                            pt[:, 0:n, :], st[:, 0:n, :], EXP, scale=SCALE
                        )
                        for j2, j in enumerate(grp):
                            r_band = j - 4 * c
                            if r_band >= 0:
                                # diagonal 128x128 block: zero where q < k
                                blk = pt[:, j2, 128 * r_band : 128 * (r_band + 1)]
                                nc.gpsimd.affine_select(
                                    out=blk,
                                    in_=blk,
                                    compare_op=mybir.AluOpType.is_ge,
                                    fill=0.0,
                                    base=0,
                                    pattern=[[1, 128]],
                                    channel_multiplier=-1,
                                )
                            for t in range(4):
                                if j > 4 * c + t:
                                    continue
                                nc.tensor.matmul(
                                    o_ts[t // 2][:, t % 2, 0:129],
                                    lhsT=pt[:, j2, 128 * t : 128 * (t + 1)],
                                    rhs=v_aug[:, j, 0:129],
                                    start=False,
                                    stop=(j == 4 * c + t),
                                    skip_group_check=True,
                                )

                    for t in range(4):
                        o = o_ts[t // 2][:, t % 2, :]
                        r = rp.tile([128, 1], F32, tag="r")
                        nc.vector.reciprocal(r[:], o[:, 128:129])
                        ot = osb.tile([128, 128], F32, tag="ot")
                        nc.vector.tensor_scalar_mul(ot[:], o[:, 0:128], r[:])
                        r0 = q0 + t * 128
                        nc.sync.dma_start(
                            out=out[r0 : r0 + 128, h * 128 : (h + 1) * 128],
                            in_=ot[:],
                        )

    nc.compile()
    return nc


_NC_CACHE = {}


def _get_nc():
    if "nc" not in _NC_CACHE:
        _NC_CACHE["nc"] = build_attention_nc()
    return _NC_CACHE["nc"]


def shard_inputs(query, key, value):
    in_maps = []
    for i in range(N_CORES):
        in_maps.append(
            {
                "query": np.ascontiguousarray(
                    query[:, i * HEADS * D : (i + 1) * HEADS * D], dtype=np.float32
                ),
                "key": np.ascontiguousarray(key[:, i * D : (i + 1) * D], dtype=np.float32),
                "value": np.ascontiguousarray(
                    value[:, i * D : (i + 1) * D], dtype=np.float32
                ),
            }
        )
    return in_maps


def kernel(query, key, value, _trace=False):
    nc = _get_nc()
    in_maps = shard_inputs(np.asarray(query), np.asarray(key), np.asarray(value))
    res = run_bass_kernel_spmd(nc, in_maps, core_ids=list(range(N_CORES)), trace=_trace)
    full = np.concatenate([res.results[i]["out"] for i in range(N_CORES)], axis=1)
    if _trace:
        kernel.last_result = res
    return full.astype(np.float32)


# revision 6
# speedup vs baseline: 1.1943x; 1.1943x over previous
"""Causal GQA attention on 8 TRN2 NeuronCores (head-sharded, no collectives).

Problem: NUM_TOKENS=2048, NUM_HEADS=32, HEAD_DIM=128, NUM_KV_HEADS=8, causal.
Sharding: core i holds KV head i and its 4 query heads (GQA group stays
together). Each core runs an independent flash-attention-style kernel:
  ST[k,q] = K @ Q^T  (bf16 matmuls, PSUM f32)
  PT      = exp(SCALE * ST)           (ACT, no max-subtraction: scores ~N(0,1))
  O[q, d+1] += PT_block^T @ [V | 1]   (ones column yields softmax denominators)
  out     = O[:, :d] / O[:, d]
"""

import numpy as np

import concourse.bass as bass
import concourse.bacc as bacc
import concourse.mybir as mybir
from concourse.tile import TileContext
from concourse.masks import make_identity
from concourse.bass_utils import run_bass_kernel_spmd

T = 2048          # tokens
D = 128           # head dim
HEADS = 4         # query heads per core
N_CORES = 8
W = 512           # q-chunk width
NKT = T // 128    # 16 k-tiles
NC_CHUNK = T // W
SCALE = D ** -0.5
F32 = mybir.dt.float32
BF16 = mybir.dt.bfloat16
EXP = mybir.ActivationFunctionType.Exp

# band packing: k-tile band offset r -> (packed col offset, width)
BAND_SLOTS = {0: (0, 512), 1: (512, 384), 3: (896, 128), 2: (1024, 256)}
BAND_W = 1280


def build_attention_nc():
    nc = bacc.Bacc("TRN2", target_bir_lowering=False, debug=False)

    q_in = nc.declare_dram_parameter("query", [T, HEADS * D], F32, isOutput=False)
    k_in = nc.declare_dram_parameter("key", [T, D], F32, isOutput=False)
    v_in = nc.declare_dram_parameter("value", [T, D], F32, isOutput=False)
    out = nc.declare_dram_parameter("out", [T, HEADS * D], F32, isOutput=True)

    # persistent SBUF tensors
    qt_sb = nc.alloc_sbuf_tensor("qt_sb", [128, HEADS, T], BF16).ap()   # [d, h, q]
    kt_sb = nc.alloc_sbuf_tensor("kt_sb", [128, NKT, 128], BF16).ap()   # [d, kt, k]
    v_aug = nc.alloc_sbuf_tensor("v_aug", [128, NKT, 132], BF16).ap()   # [k, kt, d+1]
    ident = nc.alloc_sbuf_tensor("ident", [128, 128], F32).ap()

    with TileContext(nc) as tc:
        make_identity(nc, ident)
        nc.gpsimd.memset(v_aug[:, :, 128:129], 1.0)

        # ---- phase 0: load, transpose (fp32), cast-in-copy to bf16 ----
        with (
            tc.tile_pool(name="ld", bufs=2) as ld,
            tc.tile_pool(name="trps", bufs=4, space="PSUM") as trps,
        ):
            kf = ld.tile([128, NKT, 128], F32, tag="kf", bufs=1)
            nc.sync.dma_start(
                out=kf[:], in_=k_in.rearrange("(t p) d -> p t d", p=128)
            )
            for g in range(4):
                tr = trps.tile([128, 4, 128], F32, tag="tr", name="tr")
                for j2 in range(4):
                    nc.tensor.transpose(tr[:, j2, :], kf[:, 4 * g + j2, :], ident[:])
                nc.vector.tensor_copy(kt_sb[:, 4 * g : 4 * g + 4, :], tr[:])

            vf = ld.tile([128, NKT, 128], F32, tag="vf", bufs=1)
            nc.sync.dma_start(
                out=vf[:], in_=v_in.rearrange("(t p) d -> p t d", p=128)
            )
            nc.vector.tensor_copy(v_aug[:, :, 0:128], vf[:])

            for g in range(4):
                qf = ld.tile([128, 4, 512], F32, tag="qf")
                nc.sync.dma_start(
                    out=qf[:],
                    in_=q_in[g * 512 : (g + 1) * 512, :].rearrange(
                        "(t p) d -> p t d", p=128
                    ),
                )
                for h in range(HEADS):
                    tr = trps.tile([128, 4, 128], F32, tag="tr", name="tr")
                    for t in range(4):
                        nc.tensor.transpose(
                            tr[:, t, :], qf[:, t, h * 128 : (h + 1) * 128], ident[:]
                        )
                    nc.vector.tensor_copy(
                        qt_sb[:, h, g * 512 : (g + 1) * 512], tr[:]
                    )

        # ---- phase 1: attention ----
        with (
            tc.tile_pool(name="st", bufs=2, space="PSUM") as stp,
            tc.tile_pool(name="ops", bufs=1, space="PSUM") as ops,
            tc.tile_pool(name="pt", bufs=3) as ptp,
            tc.tile_pool(name="osb", bufs=2) as osb,
            tc.tile_pool(name="rp", bufs=4) as rp,
        ):
            for h in range(HEADS):
                for c in range(NC_CHUNK):
                    q0 = c * W
                    o_ts = [
                        ops.tile([128, 2, 130], F32, tag="o01", name="o01"),
                        ops.tile([128, 2, 130], F32, tag="o23", name="o23"),
                    ]
                    nc.vector.memset(o_ts[0][:], 0.0)
                    nc.vector.memset(o_ts[1][:], 0.0)

                    def pv(j, t, lhsT):
                        nc.tensor.matmul(
                            o_ts[t // 2][:, t % 2, 0:129],
                            lhsT=lhsT,
                            rhs=v_aug[:, j, 0:129],
                            start=False,
                            stop=(j == 4 * c + t),
                            skip_group_check=True,
                        )

                    # full k-tiles, groups of 3
                    for g0 in range(0, 4 * c, 3):
                        grp = list(range(g0, min(g0 + 3, 4 * c)))
                        n = len(grp)
                        st = stp.tile([128, 3, 512], F32, tag="st", name="st")
                        pt = ptp.tile([128, 3, 512], BF16, tag="pt", name="pt")
                        for j2, j in enumerate(grp):
                            nc.tensor.matmul(
                                st[:, j2, :],
                                lhsT=kt_sb[:, j, :],
                                rhs=qt_sb[:, h, q0 : q0 + W],
                                start=True,
                                stop=True,
                            )
                        nc.scalar.activation(
                            pt[:, 0:n, :], st[:, 0:n, :], EXP, scale=SCALE
                        )
                        for j2, j in enumerate(grp):
                            for t in range(4):
                                pv(j, t, pt[:, j2, 128 * t : 128 * (t + 1)])

                    # band k-tiles (4c..4c+3), packed exact-width layout
                    stb = stp.tile([128, BAND_W], F32, tag="st", name="stb")
                    ptb = ptp.tile([128, BAND_W], BF16, tag="pt", name="ptb")
                    for r in range(4):
                        j = 4 * c + r
                        off, wd = BAND_SLOTS[r]
                        nc.tensor.matmul(
                            stb[:, off : off + wd],
                            lhsT=kt_sb[:, j, :],
                            rhs=qt_sb[:, h, q0 + 128 * r : q0 + W],
                            start=True,
                            stop=True,
                        )
                    nc.scalar.activation(ptb[:], stb[:], EXP, scale=SCALE)
                    for r in range(4):
                        j = 4 * c + r
                        off, wd = BAND_SLOTS[r]
                        # diagonal 128x128 block: zero where q < k
                        blk = ptb[:, off : off + 128]
                        nc.gpsimd.affine_select(
                            out=blk,
                            in_=blk,
                            compare_op=mybir.AluOpType.is_ge,
                            fill=0.0,
                            base=0,
                            pattern=[[1, 128]],
                            channel_multiplier=-1,
                        )
                        for t in range(r, 4):
                            boff = off + 128 * (t - r)
                            pv(j, t, ptb[:, boff : boff + 128])

                    ot = osb.tile([128, 4, 128], F32, tag="ot", name="ot")
                    for t in range(4):
                        o = o_ts[t // 2][:, t % 2, :]
                        r = rp.tile([128, 1], F32, tag="r", name="r")
                        nc.vector.reciprocal(r[:], o[:, 128:129])
                        nc.vector.tensor_scalar_mul(ot[:, t, :], o[:, 0:128], r[:])
                    nc.sync.dma_start(
                        out=out[q0 : q0 + W, h * 128 : (h + 1) * 128].rearrange(
                            "(t p) d -> p t d", p=128
                        ),
                        in_=ot[:],
                    )

    nc.compile()
    return nc


_NC_CACHE = {}


def _get_nc():
    if "nc" not in _NC_CACHE:
        _NC_CACHE["nc"] = build_attention_nc()
    return _NC_CACHE["nc"]


def shard_inputs(query, key, value):
    in_maps = []
    for i in range(N_CORES):
        in_maps.append(
            {
                "query": np.ascontiguousarray(
                    query[:, i * HEADS * D : (i + 1) * HEADS * D], dtype=np.float32
                ),
                "key": np.ascontiguousarray(key[:, i * D : (i + 1) * D], dtype=np.float32),
                "value": np.ascontiguousarray(
                    value[:, i * D : (i + 1) * D], dtype=np.float32
                ),
            }
        )
    return in_maps


def kernel(query, key, value, _trace=False):
    nc = _get_nc()
    in_maps = shard_inputs(np.asarray(query), np.asarray(key), np.asarray(value))
    res = run_bass_kernel_spmd(nc, in_maps, core_ids=list(range(N_CORES)), trace=_trace)
    full = np.concatenate([res.results[i]["out"] for i in range(N_CORES)], axis=1)
    if _trace:
        kernel.last_result = res
    return full.astype(np.float32)


# revision 8
# speedup vs baseline: 1.2126x; 1.0153x over previous
"""Causal GQA attention on 8 TRN2 NeuronCores (head-sharded, no collectives).

Problem: NUM_TOKENS=2048, NUM_HEADS=32, HEAD_DIM=128, NUM_KV_HEADS=8, causal.
Sharding: core i holds KV head i and its 4 query heads (GQA group stays
together). Each core runs an independent flash-attention-style kernel:
  ST[k,q] = K @ Q^T  (bf16 matmuls, PSUM f32)
  PT      = exp(SCALE * ST)           (ACT, no max-subtraction: scores ~N(0,1))
  O[q, d+1] += PT_block^T @ [V | 1]   (ones column yields softmax denominators)
  out     = O[:, :d] / O[:, d]
"""

import numpy as np

import concourse.bass as bass
import concourse.bacc as bacc
import concourse.mybir as mybir
from concourse.tile import TileContext
from concourse.masks import make_identity
from concourse.bass_utils import run_bass_kernel_spmd

T = 2048          # tokens
D = 128           # head dim
HEADS = 4         # query heads per core
N_CORES = 8
W = 512           # q-chunk width
NKT = T // 128    # 16 k-tiles
NC_CHUNK = T // W
SCALE = D ** -0.5
F32 = mybir.dt.float32
BF16 = mybir.dt.bfloat16
EXP = mybir.ActivationFunctionType.Exp

# band packing: k-tile band offset r -> (packed col offset, width)
BAND_SLOTS = {0: (0, 512), 1: (512, 384), 3: (896, 128), 2: (1024, 256)}
BAND_W = 1280


def build_attention_nc():
    nc = bacc.Bacc("TRN2", target_bir_lowering=False, debug=False)

    q_in = nc.declare_dram_parameter("query", [T, HEADS * D], F32, isOutput=False)
    k_in = nc.declare_dram_parameter("key", [T, D], F32, isOutput=False)
    v_in = nc.declare_dram_parameter("value", [T, D], F32, isOutput=False)
    out = nc.declare_dram_parameter("out", [T, HEADS * D], F32, isOutput=True)

    # persistent SBUF tensors
    qt_sb = nc.alloc_sbuf_tensor("qt_sb", [128, HEADS, T], BF16).ap()   # [d, h, q]
    kt_sb = nc.alloc_sbuf_tensor("kt_sb", [128, NKT, 128], BF16).ap()   # [d, kt, k]
    v_aug = nc.alloc_sbuf_tensor("v_aug", [128, NKT, 132], BF16).ap()   # [k, kt, d+1]
    ident = nc.alloc_sbuf_tensor("ident", [128, 128], F32).ap()

    with TileContext(nc) as tc:
        make_identity(nc, ident)
        nc.gpsimd.memset(v_aug[:, :, 128:129], 1.0)

        with (
            tc.tile_pool(name="ld", bufs=2) as ld,
            tc.tile_pool(name="st", bufs=2, space="PSUM") as stp,
            tc.tile_pool(name="ops", bufs=1, space="PSUM") as ops,
            tc.tile_pool(name="pt", bufs=3) as ptp,
            tc.tile_pool(name="osb", bufs=2) as osb,
            tc.tile_pool(name="rp", bufs=4) as rp,
        ):

            def load_group(g):
                # K tile-group g: k-tiles 4g..4g+3, transposed into kt_sb
                kf = ld.tile([128, 4, 128], F32, tag="kf", name="kf")
                nc.sync.dma_start(
                    out=kf[:],
                    in_=k_in[g * 512 : (g + 1) * 512, :].rearrange(
                        "(t p) d -> p t d", p=128
                    ),
                )
                # Q tile-group g for all heads
                qf = ld.tile([128, 4, 512], F32, tag="qf", name="qf")
                nc.sync.dma_start(
                    out=qf[:],
                    in_=q_in[g * 512 : (g + 1) * 512, :].rearrange(
                        "(t p) d -> p t d", p=128
                    ),
                )
                tr = stp.tile([128, 4, 128], F32, tag="st", name="tr")
                for j2 in range(4):
                    nc.tensor.transpose(tr[:, j2, :], kf[:, j2, :], ident[:])
                nc.vector.tensor_copy(kt_sb[:, 4 * g : 4 * g + 4, :], tr[:])
                for h in range(HEADS):
                    tr = stp.tile([128, 4, 128], F32, tag="st", name="tr")
                    for t in range(4):
                        nc.tensor.transpose(
                            tr[:, t, :], qf[:, t, h * 128 : (h + 1) * 128], ident[:]
                        )
                    nc.vector.tensor_copy(
                        qt_sb[:, h, g * 512 : (g + 1) * 512], tr[:]
                    )
                if g == 0:
                    vf = ld.tile([128, NKT, 128], F32, tag="vf", bufs=1, name="vf")
                    nc.sync.dma_start(
                        out=vf[:], in_=v_in.rearrange("(t p) d -> p t d", p=128)
                    )
                    nc.vector.tensor_copy(v_aug[:, :, 0:128], vf[:])

            for c in range(NC_CHUNK):
                load_group(c)
                for h in range(HEADS):
                    q0 = c * W
                    o_ts = [
                        ops.tile([128, 2, 130], F32, tag="o01", name="o01"),
                        ops.tile([128, 2, 130], F32, tag="o23", name="o23"),
                    ]
                    nc.vector.memset(o_ts[0][:], 0.0)
                    nc.vector.memset(o_ts[1][:], 0.0)

                    def pv(j, t, lhsT):
                        nc.tensor.matmul(
                            o_ts[t // 2][:, t % 2, 0:129],
                            lhsT=lhsT,
                            rhs=v_aug[:, j, 0:129],
                            start=False,
                            stop=(j == 4 * c + t),
                            skip_group_check=True,
                        )

                    # full k-tiles, groups of 3
                    for g0 in range(0, 4 * c, 3):
                        grp = list(range(g0, min(g0 + 3, 4 * c)))
                        n = len(grp)
                        st = stp.tile([128, 3, 512], F32, tag="st", name="st")
                        pt = ptp.tile([128, 3, 512], BF16, tag="pt", name="pt")
                        for j2, j in enumerate(grp):
                            nc.tensor.matmul(
                                st[:, j2, :],
                                lhsT=kt_sb[:, j, :],
                                rhs=qt_sb[:, h, q0 : q0 + W],
                                start=True,
                                stop=True,
                            )
                        nc.scalar.activation(
                            pt[:, 0:n, :], st[:, 0:n, :], EXP, scale=SCALE
                        )
                        for j2, j in enumerate(grp):
                            for t in range(4):
                                pv(j, t, pt[:, j2, 128 * t : 128 * (t + 1)])

                    # band k-tiles (4c..4c+3), packed exact-width layout
                    stb = stp.tile([128, BAND_W], F32, tag="st", name="stb")
                    ptb = ptp.tile([128, BAND_W], BF16, tag="pt", name="ptb")
                    for r in range(4):
                        j = 4 * c + r
                        off, wd = BAND_SLOTS[r]
                        nc.tensor.matmul(
                            stb[:, off : off + wd],
                            lhsT=kt_sb[:, j, :],
                            rhs=qt_sb[:, h, q0 + 128 * r : q0 + W],
                            start=True,
                            stop=True,
                        )
                    nc.scalar.activation(ptb[:], stb[:], EXP, scale=SCALE)
                    for r in range(4):
                        j = 4 * c + r
                        off, wd = BAND_SLOTS[r]
                        # diagonal 128x128 block: zero where q < k
                        blk = ptb[:, off : off + 128]
                        nc.gpsimd.affine_select(
                            out=blk,
                            in_=blk,
                            compare_op=mybir.AluOpType.is_ge,
                            fill=0.0,
                            base=0,
                            pattern=[[1, 128]],
                            channel_multiplier=-1,
                        )
                        for t in range(r, 4):
                            boff = off + 128 * (t - r)
                            pv(j, t, ptb[:, boff : boff + 128])

                    ot = osb.tile([128, 4, 128], F32, tag="ot", name="ot")
                    for t in range(4):
                        o = o_ts[t // 2][:, t % 2, :]
                        r = rp.tile([128, 1], F32, tag="r", name="r")
                        nc.vector.reciprocal(r[:], o[:, 128:129])
                        nc.vector.tensor_scalar_mul(ot[:, t, :], o[:, 0:128], r[:])
                    nc.sync.dma_start(
                        out=out[q0 : q0 + W, h * 128 : (h + 1) * 128].rearrange(
                            "(t p) d -> p t d", p=128
                        ),
                        in_=ot[:],
                    )

    nc.compile()
    return nc


_NC_CACHE = {}


def _get_nc():
    if "nc" not in _NC_CACHE:
        _NC_CACHE["nc"] = build_attention_nc()
    return _NC_CACHE["nc"]


def shard_inputs(query, key, value):
    in_maps = []
    for i in range(N_CORES):
        in_maps.append(
            {
                "query": np.ascontiguousarray(
                    query[:, i * HEADS * D : (i + 1) * HEADS * D], dtype=np.float32
                ),
                "key": np.ascontiguousarray(key[:, i * D : (i + 1) * D], dtype=np.float32),
                "value": np.ascontiguousarray(
                    value[:, i * D : (i + 1) * D], dtype=np.float32
                ),
            }
        )
    return in_maps


def kernel(query, key, value, _trace=False):
    nc = _get_nc()
    in_maps = shard_inputs(np.asarray(query), np.asarray(key), np.asarray(value))
    res = run_bass_kernel_spmd(nc, in_maps, core_ids=list(range(N_CORES)), trace=_trace)
    full = np.concatenate([res.results[i]["out"] for i in range(N_CORES)], axis=1)
    if _trace:
        kernel.last_result = res
    return full.astype(np.float32)


# revision 9
# speedup vs baseline: 1.2212x; 1.0071x over previous
"""Causal GQA attention on 8 TRN2 NeuronCores (head-sharded, no collectives).

Problem: NUM_TOKENS=2048, NUM_HEADS=32, HEAD_DIM=128, NUM_KV_HEADS=8, causal.
Sharding: core i holds KV head i and its 4 query heads (GQA group stays
together). Each core runs an independent flash-attention-style kernel:
  ST[k,q] = K @ Q^T  (bf16 matmuls, PSUM f32)
  PT      = exp(SCALE * ST)           (ACT, no max-subtraction: scores ~N(0,1))
  O[q, d+1] += PT_block^T @ [V | 1]   (ones column yields softmax denominators)
  out     = O[:, :d] / O[:, d]
"""

import numpy as np

import concourse.bass as bass
import concourse.bacc as bacc
import concourse.mybir as mybir
from concourse.tile import TileContext
from concourse.masks import make_identity
from concourse.bass_utils import run_bass_kernel_spmd

T = 2048          # tokens
D = 128           # head dim
HEADS = 4         # query heads per core
N_CORES = 8
W = 512           # q-chunk width
NKT = T // 128    # 16 k-tiles
NC_CHUNK = T // W
SCALE = D ** -0.5
F32 = mybir.dt.float32
BF16 = mybir.dt.bfloat16
EXP = mybir.ActivationFunctionType.Exp

# band packing: k-tile band offset r -> (packed col offset, width)
BAND_SLOTS = {0: (0, 512), 1: (512, 384), 3: (896, 128), 2: (1024, 256)}
BAND_W = 1280


def build_attention_nc():
    nc = bacc.Bacc("TRN2", target_bir_lowering=False, debug=False)

    q_in = nc.declare_dram_parameter("query", [T, HEADS * D], F32, isOutput=False)
    k_in = nc.declare_dram_parameter("key", [T, D], F32, isOutput=False)
    v_in = nc.declare_dram_parameter("value", [T, D], F32, isOutput=False)
    out = nc.declare_dram_parameter("out", [T, HEADS * D], F32, isOutput=True)

    # persistent SBUF tensors
    qt_sb = nc.alloc_sbuf_tensor("qt_sb", [128, HEADS, T], BF16).ap()   # [d, h, q]
    kt_sb = nc.alloc_sbuf_tensor("kt_sb", [128, NKT, 128], BF16).ap()   # [d, kt, k]
    v_aug = nc.alloc_sbuf_tensor("v_aug", [128, NKT, 132], BF16).ap()   # [k, kt, d+1]
    ident = nc.alloc_sbuf_tensor("ident", [128, 128], F32).ap()

    with TileContext(nc) as tc:
        make_identity(nc, ident)
        nc.gpsimd.memset(v_aug[:, :, 128:129], 1.0)

        with (
            tc.tile_pool(name="ld", bufs=2) as ld,
            tc.tile_pool(name="st", bufs=2, space="PSUM") as stp,
            tc.tile_pool(name="ops", bufs=1, space="PSUM") as ops,
            tc.tile_pool(name="pt", bufs=3) as ptp,
            tc.tile_pool(name="osb", bufs=2) as osb,
            tc.tile_pool(name="rp", bufs=4) as rp,
        ):

            def load_group(g):
                rows = slice(g * 512, (g + 1) * 512)
                # K tile-group g: k-tiles 4g..4g+3, transposed into kt_sb
                kf = ld.tile([128, 4, 128], F32, tag="kf", name="kf")
                if g == 0:
                    # group 0 is the critical path: per-head Q loads so head 0
                    # can start as soon as its own 256KB arrives
                    qfs = []
                    for h in range(HEADS):
                        qfh = ld.tile([128, 4, 128], F32, tag="qf0", bufs=4, name="qfh")
                        nc.sync.dma_start(
                            out=qfh[:],
                            in_=q_in[rows, h * 128 : (h + 1) * 128].rearrange(
                                "(t p) d -> p t d", p=128
                            ),
                        )
                        qfs.append(qfh)
                        if h == 0:
                            nc.sync.dma_start(
                                out=kf[:],
                                in_=k_in[rows, :].rearrange("(t p) d -> p t d", p=128),
                            )
                    q_slice = lambda h, t: qfs[h][:, t, :]
                else:
                    nc.sync.dma_start(
                        out=kf[:],
                        in_=k_in[rows, :].rearrange("(t p) d -> p t d", p=128),
                    )
                    qf = ld.tile([128, 4, 512], F32, tag="qf", name="qf")
                    nc.sync.dma_start(
                        out=qf[:],
                        in_=q_in[rows, :].rearrange("(t p) d -> p t d", p=128),
                    )
                    q_slice = lambda h, t: qf[:, t, h * 128 : (h + 1) * 128]
                tr = stp.tile([128, 4, 128], F32, tag="st", name="tr")
                for j2 in range(4):
                    nc.tensor.transpose(tr[:, j2, :], kf[:, j2, :], ident[:])
                nc.vector.tensor_copy(kt_sb[:, 4 * g : 4 * g + 4, :], tr[:])
                for h in range(HEADS):
                    tr = stp.tile([128, 4, 128], F32, tag="st", name="tr")
                    for t in range(4):
                        nc.tensor.transpose(tr[:, t, :], q_slice(h, t), ident[:])
                    nc.vector.tensor_copy(
                        qt_sb[:, h, g * 512 : (g + 1) * 512], tr[:]
                    )
                if g == 0:
                    vf = ld.tile([128, NKT, 128], F32, tag="vf", bufs=1, name="vf")
                    nc.sync.dma_start(
                        out=vf[:], in_=v_in.rearrange("(t p) d -> p t d", p=128)
                    )
                    nc.vector.tensor_copy(v_aug[:, :, 0:128], vf[:])

            load_group(0)
            for c in range(NC_CHUNK):
                for h in range(HEADS):
                    if h == 2 and c + 1 < NC_CHUNK:
                        load_group(c + 1)
                    q0 = c * W
                    o_ts = [
                        ops.tile([128, 2, 130], F32, tag="o01", name="o01"),
                        ops.tile([128, 2, 130], F32, tag="o23", name="o23"),
                    ]
                    nc.vector.memset(o_ts[0][:], 0.0)
                    nc.vector.memset(o_ts[1][:], 0.0)

                    def pv(j, t, lhsT):
                        nc.tensor.matmul(
                            o_ts[t // 2][:, t % 2, 0:129],
                            lhsT=lhsT,
                            rhs=v_aug[:, j, 0:129],
                            start=False,
                            stop=(j == 4 * c + t),
                            skip_group_check=True,
                        )

                    # full k-tiles, groups of 3
                    for g0 in range(0, 4 * c, 3):
                        grp = list(range(g0, min(g0 + 3, 4 * c)))
                        n = len(grp)
                        st = stp.tile([128, 3, 512], F32, tag="st", name="st")
                        pt = ptp.tile([128, 3, 512], BF16, tag="pt", name="pt")
                        for j2, j in enumerate(grp):
                            nc.tensor.matmul(
                                st[:, j2, :],
                                lhsT=kt_sb[:, j, :],
                                rhs=qt_sb[:, h, q0 : q0 + W],
                                start=True,
                                stop=True,
                            )
                        nc.scalar.activation(
                            pt[:, 0:n, :], st[:, 0:n, :], EXP, scale=SCALE
                        )
                        for j2, j in enumerate(grp):
                            for t in range(4):
                                pv(j, t, pt[:, j2, 128 * t : 128 * (t + 1)])

                    # band k-tiles (4c..4c+3), packed exact-width layout
                    stb = stp.tile([128, BAND_W], F32, tag="st", name="stb")
                    ptb = ptp.tile([128, BAND_W], BF16, tag="pt", name="ptb")
                    for r in range(4):
                        j = 4 * c + r
                        off, wd = BAND_SLOTS[r]
                        nc.tensor.matmul(
                            stb[:, off : off + wd],
                            lhsT=kt_sb[:, j, :],
                            rhs=qt_sb[:, h, q0 + 128 * r : q0 + W],
                            start=True,
                            stop=True,
                        )
                    nc.scalar.activation(ptb[:], stb[:], EXP, scale=SCALE)
                    for r in range(4):
                        j = 4 * c + r
                        off, wd = BAND_SLOTS[r]
                        # diagonal 128x128 block: zero where q < k
                        blk = ptb[:, off : off + 128]
                        nc.gpsimd.affine_select(
                            out=blk,
                            in_=blk,
                            compare_op=mybir.AluOpType.is_ge,
                            fill=0.0,
                            base=0,
                            pattern=[[1, 128]],
                            channel_multiplier=-1,
                        )
                        for t in range(r, 4):
                            boff = off + 128 * (t - r)
                            pv(j, t, ptb[:, boff : boff + 128])

                    ot = osb.tile([128, 4, 128], F32, tag="ot", name="ot")
                    for t in range(4):
                        o = o_ts[t // 2][:, t % 2, :]
                        r = rp.tile([128, 1], F32, tag="r", name="r")
                        nc.vector.reciprocal(r[:], o[:, 128:129])
                        nc.vector.tensor_scalar_mul(ot[:, t, :], o[:, 0:128], r[:])
                    nc.sync.dma_start(
                        out=out[q0 : q0 + W, h * 128 : (h + 1) * 128].rearrange(
                            "(t p) d -> p t d", p=128
                        ),
                        in_=ot[:],
                    )

    nc.compile()
    return nc


_NC_CACHE = {}


def _get_nc():
    if "nc" not in _NC_CACHE:
        _NC_CACHE["nc"] = build_attention_nc()
    return _NC_CACHE["nc"]


def shard_inputs(query, key, value):
    in_maps = []
    for i in range(N_CORES):
        in_maps.append(
            {
                "query": np.ascontiguousarray(
                    query[:, i * HEADS * D : (i + 1) * HEADS * D], dtype=np.float32
                ),
                "key": np.ascontiguousarray(key[:, i * D : (i + 1) * D], dtype=np.float32),
                "value": np.ascontiguousarray(
                    value[:, i * D : (i + 1) * D], dtype=np.float32
                ),
            }
        )
    return in_maps


def kernel(query, key, value, _trace=False):
    nc = _get_nc()
    in_maps = shard_inputs(np.asarray(query), np.asarray(key), np.asarray(value))
    res = run_bass_kernel_spmd(nc, in_maps, core_ids=list(range(N_CORES)), trace=_trace)
    full = np.concatenate([res.results[i]["out"] for i in range(N_CORES)], axis=1)
    if _trace:
        kernel.last_result = res
    return full.astype(np.float32)


# revision 13
# speedup vs baseline: 1.2885x; 1.0551x over previous
"""Causal GQA attention on 8 TRN2 NeuronCores (head-sharded, no collectives).

Problem: NUM_TOKENS=2048, NUM_HEADS=32, HEAD_DIM=128, NUM_KV_HEADS=8, causal.
Sharding: core i holds KV head i and its 4 query heads (GQA group stays
together). Each core runs an independent flash-attention-style kernel:
  ST[k,q] = K @ Q^T  (bf16 matmuls, PSUM f32)
  PT      = exp(SCALE * ST)           (ACT, no max-subtraction: scores ~N(0,1))
  O[q, d+1] += PT_block^T @ [V | 1]   (ones column yields softmax denominators)
  out     = O[:, :d] / O[:, d]
The emission is software-pipelined: score matmuls of group i+1 are emitted
before the PV matmuls of group i, so the PE computes scores while ACT
exponentiates and never idles waiting for the activation.
"""

import numpy as np

import concourse.bass as bass
import concourse.bacc as bacc
import concourse.mybir as mybir
from concourse.tile import TileContext
from concourse.masks import make_identity
from concourse.bass_utils import run_bass_kernel_spmd

T = 2048          # tokens
D = 128           # head dim
HEADS = 4         # query heads per core
N_CORES = 8
W = 512           # q-chunk width
NKT = T // 128    # 16 k-tiles
NC_CHUNK = T // W
SCALE = D ** -0.5
F32 = mybir.dt.float32
BF16 = mybir.dt.bfloat16
EXP = mybir.ActivationFunctionType.Exp

# band packing: k-tile band offset r -> (packed col offset, width)
BAND_SLOTS = {0: (0, 512), 1: (512, 384), 3: (896, 128), 2: (1024, 256)}
BAND_W = 1280


def build_attention_nc():
    nc = bacc.Bacc("TRN2", target_bir_lowering=False, debug=False)

    q_in = nc.declare_dram_parameter("query", [T, HEADS * D], F32, isOutput=False)
    k_in = nc.declare_dram_parameter("key", [T, D], F32, isOutput=False)
    v_in = nc.declare_dram_parameter("value", [T, D], F32, isOutput=False)
    out = nc.declare_dram_parameter("out", [T, HEADS * D], F32, isOutput=True)

    # persistent SBUF tensors
    qt_sb = nc.alloc_sbuf_tensor("qt_sb", [128, HEADS, T], BF16).ap()   # [d, h, q]
    kt_sb = nc.alloc_sbuf_tensor("kt_sb", [128, NKT, 128], BF16).ap()   # [d, kt, k]
    v_aug = nc.alloc_sbuf_tensor("v_aug", [128, NKT, 132], BF16).ap()   # [k, kt, d+1]
    ident = nc.alloc_sbuf_tensor("ident", [128, 128], BF16).ap()

    with TileContext(nc) as tc:
        make_identity(nc, ident)
        nc.gpsimd.memset(v_aug[:, :, 128:129], 1.0)

        with (
            tc.tile_pool(name="ld", bufs=2) as ld,
            tc.tile_pool(name="ldb", bufs=2) as ldb,
            tc.tile_pool(name="st", bufs=2, space="PSUM") as stp,
            tc.tile_pool(name="ops", bufs=1, space="PSUM") as ops,
            tc.tile_pool(name="pt", bufs=3) as ptp,
            tc.tile_pool(name="osb", bufs=2) as osb,
            tc.tile_pool(name="rp", bufs=4) as rp,
        ):
            loaded = {}

            def load_group_dma(g):
                """Dispatch the DMAs for Q/K tile-group g (k/q-tiles 4g..4g+3)."""
                rows = slice(g * 512, (g + 1) * 512)
                kf = ld.tile([128, 4, 128], F32, tag="kf", name="kf")
                if g == 0:
                    # group 0 is the critical path: per-head Q loads so head 0
                    # can start as soon as its own 256KB arrives
                    qfs = []
                    for h in range(HEADS):
                        qfh = ld.tile([128, 4, 128], F32, tag="qf0", bufs=4, name="qfh")
                        nc.sync.dma_start(
                            out=qfh[:],
                            in_=q_in[rows, h * 128 : (h + 1) * 128].rearrange(
                                "(t p) d -> p t d", p=128
                            ),
                        )
                        qfs.append(qfh)
                        if h == 0:
                            nc.sync.dma_start(
                                out=kf[:],
                                in_=k_in[rows, :].rearrange("(t p) d -> p t d", p=128),
                            )
                    q_slice = lambda h: qfs[h][:, :, :]
                else:
                    nc.sync.dma_start(
                        out=kf[:],
                        in_=k_in[rows, :].rearrange("(t p) d -> p t d", p=128),
                    )
                    qf = ld.tile([128, 4, 512], F32, tag="qf", name="qf")
                    nc.sync.dma_start(
                        out=qf[:],
                        in_=q_in[rows, :].rearrange("(t p) d -> p t d", p=128),
                    )
                    q_slice = lambda h: qf[:, :, h * 128 : (h + 1) * 128]
                loaded[g] = (kf, q_slice)
                if g == 0:
                    vf = ld.tile([128, NKT, 128], F32, tag="vf", bufs=1, name="vf")
                    nc.sync.dma_start(
                        out=vf[:], in_=v_in.rearrange("(t p) d -> p t d", p=128)
                    )
                    nc.vector.tensor_copy(v_aug[:, :, 0:128], vf[:])

            def load_group_tr(g):
                """Cast to bf16 and PE-transpose group g into kt_sb / qt_sb."""
                kf, q_slice = loaded.pop(g)
                kb = ldb.tile([128, 4, 128], BF16, tag="kb", name="kb")
                nc.vector.tensor_copy(kb[:], kf[:])
                tr = stp.tile([128, 4, 128], BF16, tag="st", name="tr")
                for j2 in range(4):
                    nc.tensor.transpose(tr[:, j2, :], kb[:, j2, :], ident[:])
                nc.vector.tensor_copy(kt_sb[:, 4 * g : 4 * g + 4, :], tr[:])
                for h in range(HEADS):
                    qb = ldb.tile([128, 4, 128], BF16, tag="qb", name="qb")
                    nc.vector.tensor_copy(qb[:], q_slice(h))
                    tr = stp.tile([128, 4, 128], BF16, tag="st", name="tr")
                    for t in range(4):
                        nc.tensor.transpose(tr[:, t, :], qb[:, t, :], ident[:])
                    nc.vector.tensor_copy(
                        qt_sb[:, h, g * 512 : (g + 1) * 512], tr[:]
                    )

            # ---- software-pipelined attention ----
            pending = [None]

            def flush():
                if pending[0] is not None:
                    pending[0]()
                    pending[0] = None

            def attn_head(h, c):
                q0 = c * W
                state = {}
                n_grp_total = (4 * c + 2) // 3 + 1  # full groups + band
                grp_idx = [0]

                def ensure_o():
                    if "o" in state:
                        return
                    o_ts = [
                        ops.tile([128, 2, 130], F32, tag="o01", name="o01"),
                        ops.tile([128, 2, 130], F32, tag="o23", name="o23"),
                    ]
                    nc.vector.memset(o_ts[0][:], 0.0)
                    nc.vector.memset(o_ts[1][:], 0.0)
                    state["o"] = o_ts

                def pv(j, t, lhsT):
                    nc.tensor.matmul(
                        state["o"][t // 2][:, t % 2, 0:129],
                        lhsT=lhsT,
                        rhs=v_aug[:, j, 0:129],
                        start=False,
                        stop=(j == 4 * c + t),
                        skip_group_check=True,
                    )

                def finish():
                    o_ts = state["o"]
                    ot = osb.tile([128, 4, 128], F32, tag="ot", name="ot")
                    for t in range(4):
                        o = o_ts[t // 2][:, t % 2, :]
                        r = rp.tile([128, 1], F32, tag="r", name="r")
                        nc.vector.reciprocal(r[:], o[:, 128:129])
                        nc.vector.tensor_scalar_mul(ot[:, t, :], o[:, 0:128], r[:])
                    nc.sync.dma_start(
                        out=out[q0 : q0 + W, h * 128 : (h + 1) * 128].rearrange(
                            "(t p) d -> p t d", p=128
                        ),
                        in_=ot[:],
                    )

                # full k-tiles, groups of 3
                for g0 in range(0, 4 * c, 3):
                    grp = list(range(g0, min(g0 + 3, 4 * c)))
                    n = len(grp)
                    st = stp.tile([128, 3, 512], F32, tag="st", name="st")
                    pt = ptp.tile([128, 3, 512], BF16, tag="pt", name="pt")
                    for j2, j in enumerate(grp):
                        nc.tensor.matmul(
                            st[:, j2, :],
                            lhsT=kt_sb[:, j, :],
                            rhs=qt_sb[:, h, q0 : q0 + W],
                            start=True,
                            stop=True,
                        )
                    nc.scalar.activation(pt[:, 0:n, :], st[:, 0:n, :], EXP, scale=SCALE)
                    flush()

                    def emit_full(grp=grp, pt=pt):
                        ensure_o()
                        for j2, j in enumerate(grp):
                            for t in range(4):
                                pv(j, t, pt[:, j2, 128 * t : 128 * (t + 1)])

                    pending[0] = emit_full

                # band k-tiles (4c..4c+3), packed exact-width layout
                stb = stp.tile([128, BAND_W], F32, tag="st", name="stb")
                ptb = ptp.tile([128, BAND_W], BF16, tag="pt", name="ptb")
                for r in range(4):
                    j = 4 * c + r
                    off, wd = BAND_SLOTS[r]
                    nc.tensor.matmul(
                        stb[:, off : off + wd],
                        lhsT=kt_sb[:, j, :],
                        rhs=qt_sb[:, h, q0 + 128 * r : q0 + W],
                        start=True,
                        stop=True,
                    )
                nc.scalar.activation(ptb[:], stb[:], EXP, scale=SCALE)
                flush()

                def emit_band(ptb=ptb):
                    ensure_o()
                    for r in range(4):
                        j = 4 * c + r
                        off, wd = BAND_SLOTS[r]
                        # diagonal 128x128 block: zero where q < k
                        blk = ptb[:, off : off + 128]
                        nc.gpsimd.affine_select(
                            out=blk,
                            in_=blk,
                            compare_op=mybir.AluOpType.is_ge,
                            fill=0.0,
                            base=0,
                            pattern=[[1, 128]],
                            channel_multiplier=-1,
                        )
                        for t in range(r, 4):
                            boff = off + 128 * (t - r)
                            pv(j, t, ptb[:, boff : boff + 128])
                    finish()

                pending[0] = emit_band

            load_group_dma(0)
            load_group_tr(0)
            for c in range(NC_CHUNK):
                for h in range(HEADS):
                    if h == 0 and c + 1 < NC_CHUNK:
                        load_group_dma(c + 1)
                    if h == 3 and c + 1 < NC_CHUNK:
                        load_group_tr(c + 1)
                    attn_head(h, c)
            flush()

    nc.compile()
    return nc


_NC_CACHE = {}


def _get_nc():
    if "nc" not in _NC_CACHE:
        _NC_CACHE["nc"] = build_attention_nc()
    return _NC_CACHE["nc"]


def shard_inputs(query, key, value):
    in_maps = []
    for i in range(N_CORES):
        in_maps.append(
            {
                "query": np.ascontiguousarray(
                    query[:, i * HEADS * D : (i + 1) * HEADS * D], dtype=np.float32
                ),
                "key": np.ascontiguousarray(key[:, i * D : (i + 1) * D], dtype=np.float32),
                "value": np.ascontiguousarray(
                    value[:, i * D : (i + 1) * D], dtype=np.float32
                ),
            }
        )
    return in_maps


def kernel(query, key, value, _trace=False):
    nc = _get_nc()
    in_maps = shard_inputs(np.asarray(query), np.asarray(key), np.asarray(value))
    res = run_bass_kernel_spmd(nc, in_maps, core_ids=list(range(N_CORES)), trace=_trace)
    full = np.concatenate([res.results[i]["out"] for i in range(N_CORES)], axis=1)
    if _trace:
        kernel.last_result = res
    return full.astype(np.float32)


# revision 15
# speedup vs baseline: 1.3326x; 1.0343x over previous
"""Causal GQA attention on 8 TRN2 NeuronCores (head-sharded, no collectives).

Problem: NUM_TOKENS=2048, NUM_HEADS=32, HEAD_DIM=128, NUM_KV_HEADS=8, causal.
Sharding: core i holds KV head i and its 4 query heads (GQA group stays
together). Each core runs an independent flash-attention-style kernel:
  ST[k,q] = K @ Q^T  (bf16 matmuls, PSUM f32)
  PT      = exp(SCALE * ST)           (ACT, no max-subtraction: scores ~N(0,1))
  O[q, d+1] += PT_block^T @ [V | 1]   (ones column yields softmax denominators)
  out     = O[:, :d] / O[:, d]
The emission is software-pipelined: score matmuls of group i+1 are emitted
before the PV matmuls of group i, so the PE computes scores while ACT
exponentiates and never idles waiting for the activation.
"""

import numpy as np

import concourse.bass as bass
import concourse.bacc as bacc
import concourse.mybir as mybir
from concourse.tile import TileContext
from concourse.masks import make_identity
from concourse.bass_utils import run_bass_kernel_spmd

T = 2048          # tokens
D = 128           # head dim
HEADS = 4         # query heads per core
N_CORES = 8
W = 512           # q-chunk width
NKT = T // 128    # 16 k-tiles
NC_CHUNK = T // W
SCALE = D ** -0.5
F32 = mybir.dt.float32
BF16 = mybir.dt.bfloat16
EXP = mybir.ActivationFunctionType.Exp

# band packing: k-tile band offset r -> (packed col offset, width)
BAND_SLOTS = {0: (0, 512), 1: (512, 384), 3: (896, 128), 2: (1024, 256)}
BAND_W = 1280


def build_attention_nc():
    nc = bacc.Bacc("TRN2", target_bir_lowering=False, debug=False)

    q_in = nc.declare_dram_parameter("query", [T, HEADS * D], F32, isOutput=False)
    k_in = nc.declare_dram_parameter("key", [T, D], F32, isOutput=False)
    v_in = nc.declare_dram_parameter("value", [T, D], F32, isOutput=False)
    out = nc.declare_dram_parameter("out", [T, HEADS * D], F32, isOutput=True)

    # persistent SBUF tensors
    qt_sb = nc.alloc_sbuf_tensor("qt_sb", [128, HEADS, T], BF16).ap()   # [d, h, q]
    kt_sb = nc.alloc_sbuf_tensor("kt_sb", [128, NKT, 128], BF16).ap()   # [d, kt, k]
    v_aug = nc.alloc_sbuf_tensor("v_aug", [128, NKT, 132], BF16).ap()   # [k, kt, d+1]
    ident = nc.alloc_sbuf_tensor("ident", [128, 128], BF16).ap()

    with TileContext(nc) as tc:
        make_identity(nc, ident)
        nc.gpsimd.memset(v_aug[:, :, 128:129], 1.0)

        with (
            tc.tile_pool(name="ld", bufs=2) as ld,
            tc.tile_pool(name="ldb", bufs=2) as ldb,
            tc.tile_pool(name="st", bufs=2, space="PSUM") as stp,
            tc.tile_pool(name="ops", bufs=1, space="PSUM") as ops,
            tc.tile_pool(name="pt", bufs=3) as ptp,
            tc.tile_pool(name="osb", bufs=2) as osb,
            tc.tile_pool(name="rp", bufs=4) as rp,
        ):
            loaded = {}

            def load_group_dma(g):
                """Dispatch the DMAs for Q/K tile-group g (k/q-tiles 4g..4g+3)."""
                rows = slice(g * 512, (g + 1) * 512)
                kf = ld.tile([128, 4, 128], F32, tag="kf", name="kf")
                if g == 0:
                    # group 0 is the critical path: per-head Q loads so head 0
                    # can start as soon as its own 256KB arrives
                    nc.sync.dma_start(
                        out=kf[:],
                        in_=k_in[rows, :].rearrange("(t p) d -> p t d", p=128),
                    )
                    qfs = []
                    for h in range(HEADS):
                        qfh = ld.tile([128, 4, 128], F32, tag="qf0", bufs=4, name="qfh")
                        nc.sync.dma_start(
                            out=qfh[:],
                            in_=q_in[rows, h * 128 : (h + 1) * 128].rearrange(
                                "(t p) d -> p t d", p=128
                            ),
                        )
                        qfs.append(qfh)
                    q_slice = lambda h: qfs[h][:, :, :]
                else:
                    nc.sync.dma_start(
                        out=kf[:],
                        in_=k_in[rows, :].rearrange("(t p) d -> p t d", p=128),
                    )
                    qf = ld.tile([128, 4, 512], F32, tag="qf", name="qf")
                    nc.sync.dma_start(
                        out=qf[:],
                        in_=q_in[rows, :].rearrange("(t p) d -> p t d", p=128),
                    )
                    q_slice = lambda h: qf[:, :, h * 128 : (h + 1) * 128]
                loaded[g] = (kf, q_slice)
                if g == 0:
                    vf = ld.tile([128, NKT, 128], F32, tag="vf", bufs=1, name="vf")
                    nc.sync.dma_start(
                        out=vf[:], in_=v_in.rearrange("(t p) d -> p t d", p=128)
                    )
                    nc.vector.tensor_copy(v_aug[:, :, 0:128], vf[:])

            def load_group_tr(g):
                """Cast to bf16 and PE-transpose group g into kt_sb / qt_sb."""
                kf, q_slice = loaded.pop(g)
                kb = ldb.tile([128, 4, 128], BF16, tag="kb", name="kb")
                nc.vector.tensor_copy(kb[:], kf[:])
                tr = stp.tile([128, 4, 128], BF16, tag="st", name="tr")
                for j2 in range(4):
                    nc.tensor.transpose(tr[:, j2, :], kb[:, j2, :], ident[:])
                nc.vector.tensor_copy(kt_sb[:, 4 * g : 4 * g + 4, :], tr[:])
                for h in range(HEADS):
                    qb = ldb.tile([128, 4, 128], BF16, tag="qb", name="qb")
                    nc.vector.tensor_copy(qb[:], q_slice(h))
                    tr = stp.tile([128, 4, 128], BF16, tag="st", name="tr")
                    for t in range(4):
                        nc.tensor.transpose(tr[:, t, :], qb[:, t, :], ident[:])
                    nc.vector.tensor_copy(
                        qt_sb[:, h, g * 512 : (g + 1) * 512], tr[:]
                    )

            # ---- software-pipelined attention ----
            pending = [None]

            def flush():
                if pending[0] is not None:
                    pending[0]()
                    pending[0] = None

            def attn_head(h, c):
                q0 = c * W
                state = {}
                n_grp_total = (4 * c + 2) // 3 + 1  # full groups + band
                grp_idx = [0]

                def ensure_o():
                    if "o" in state:
                        return
                    state["o"] = [
                        ops.tile([128, 2, 130], F32, tag="o01", name="o01"),
                        ops.tile([128, 2, 130], F32, tag="o23", name="o23"),
                    ]
                    nc.vector.memset(state["o"][0][:], 0.0)
                    nc.vector.memset(state["o"][1][:], 0.0)

                def pv(j, t, lhsT):
                    nc.tensor.matmul(
                        state["o"][t // 2][:, t % 2, 0:129],
                        lhsT=lhsT,
                        rhs=v_aug[:, j, 0:129],
                        start=False,
                        stop=(j == 4 * c + t),
                        skip_group_check=True,
                    )

                def finish():
                    o_ts = state["o"]
                    ot = osb.tile([128, 4, 128], F32, tag="ot", name="ot")
                    for i in range(2):
                        r = rp.tile([128, 2, 1], F32, tag="r", name="r")
                        nc.vector.reciprocal(r[:], o_ts[i][:, :, 128:129])
                        nc.vector.tensor_mul(
                            ot[:, 2 * i : 2 * i + 2, :],
                            o_ts[i][:, :, 0:128],
                            r[:].broadcast_to([128, 2, 128]),
                        )
                    nc.sync.dma_start(
                        out=out[q0 : q0 + W, h * 128 : (h + 1) * 128].rearrange(
                            "(t p) d -> p t d", p=128
                        ),
                        in_=ot[:],
                    )

                # full k-tiles, groups of 3
                for g0 in range(0, 4 * c, 3):
                    grp = list(range(g0, min(g0 + 3, 4 * c)))
                    n = len(grp)
                    st = stp.tile([128, 3, 512], F32, tag="st", name="st")
                    pt = ptp.tile([128, 3, 512], BF16, tag="pt", name="pt")
                    for j2, j in enumerate(grp):
                        nc.tensor.matmul(
                            st[:, j2, :],
                            lhsT=kt_sb[:, j, :],
                            rhs=qt_sb[:, h, q0 : q0 + W],
                            start=True,
                            stop=True,
                        )
                    nc.scalar.activation(pt[:, 0:n, :], st[:, 0:n, :], EXP, scale=SCALE)
                    flush()

                    def emit_full(grp=grp, pt=pt):
                        ensure_o()
                        for j2, j in enumerate(grp):
                            for t in range(4):
                                pv(j, t, pt[:, j2, 128 * t : 128 * (t + 1)])

                    pending[0] = emit_full

                # band k-tiles (4c..4c+3), packed exact-width layout
                stb = stp.tile([128, BAND_W], F32, tag="st", name="stb")
                ptb = ptp.tile([128, BAND_W], BF16, tag="pt", name="ptb")
                for r in range(4):
                    j = 4 * c + r
                    off, wd = BAND_SLOTS[r]
                    nc.tensor.matmul(
                        stb[:, off : off + wd],
                        lhsT=kt_sb[:, j, :],
                        rhs=qt_sb[:, h, q0 + 128 * r : q0 + W],
                        start=True,
                        stop=True,
                    )
                nc.scalar.activation(ptb[:], stb[:], EXP, scale=SCALE)
                flush()

                def emit_band(ptb=ptb):
                    ensure_o()
                    for r in range(4):
                        j = 4 * c + r
                        off, wd = BAND_SLOTS[r]
                        # diagonal 128x128 block: zero where q < k
                        blk = ptb[:, off : off + 128]
                        nc.gpsimd.affine_select(
                            out=blk,
                            in_=blk,
                            compare_op=mybir.AluOpType.is_ge,
                            fill=0.0,
                            base=0,
                            pattern=[[1, 128]],
                            channel_multiplier=-1,
                        )
                        for t in range(r, 4):
                            boff = off + 128 * (t - r)
                            pv(j, t, ptb[:, boff : boff + 128])
                    finish()

                pending[0] = emit_band

            load_group_dma(0)
            load_group_tr(0)
            for c in range(NC_CHUNK):
                for h in range(HEADS):
                    if h == 0 and c + 1 < NC_CHUNK:
                        load_group_dma(c + 1)
                    if h == 1 and c + 1 < NC_CHUNK:
                        load_group_tr(c + 1)
                    attn_head(h, c)
            flush()

    nc.compile()
    return nc


_NC_CACHE = {}


def _get_nc():
    if "nc" not in _NC_CACHE:
        _NC_CACHE["nc"] = build_attention_nc()
    return _NC_CACHE["nc"]


def shard_inputs(query, key, value):
    in_maps = []
    for i in range(N_CORES):
        in_maps.append(
            {
                "query": np.ascontiguousarray(
                    query[:, i * HEADS * D : (i + 1) * HEADS * D], dtype=np.float32
                ),
                "key": np.ascontiguousarray(key[:, i * D : (i + 1) * D], dtype=np.float32),
                "value": np.ascontiguousarray(
                    value[:, i * D : (i + 1) * D], dtype=np.float32
                ),
            }
        )
    return in_maps


def kernel(query, key, value, _trace=False):
    nc = _get_nc()
    in_maps = shard_inputs(np.asarray(query), np.asarray(key), np.asarray(value))
    res = run_bass_kernel_spmd(nc, in_maps, core_ids=list(range(N_CORES)), trace=_trace)
    full = np.concatenate([res.results[i]["out"] for i in range(N_CORES)], axis=1)
    if _trace:
        kernel.last_result = res
    return full.astype(np.float32)
